# revision 3
# baseline (speedup 1.0000x reference)
"""Trainium2 Bass kernel for nn_Net_996432413190 (GNN message passing).

v1 strategy: host does index bookkeeping (edge gathers / segment means); the
8 NeuronCores run the dominant compute: three EdgeConv MLPs
(1.2M x 128 @ 128 x 64 matmul + bias + ELU + BatchNorm affine), edge-sharded
across cores (150k edges/core), feature-major. ELU is composed from
Relu/Exp ACT passes (no native Elu): elu(z) = max(z,0) + min(exp(z)-1, 0).
One NEFF compiled once, executed 3x (one per layer).
"""
import sys
sys.path.insert(0, '/opt/trn_rl_repo')
import numpy as np

import concourse.bass as bass
import concourse.mybir as mybir
from concourse import bacc
from concourse.tile import TileContext
from concourse.bass_utils import run_bass_kernel_spmd

N = 100000
E = 1200000
H = 64
NCORES = 8
EPC = E // NCORES          # 150000 edges per core
TILE = 500                 # rows per matmul (<=512 fp32 moving limit)
COLS = 5000                # rows per DMA block (128 x 5000 x 4B = 2.5MB)
BN_EPS = 1e-5

_cache = {}


def _build_nc():
    nc = bacc.Bacc("TRN2", target_bir_lowering=False)
    msg = nc.dram_tensor("msg", [128, EPC], mybir.dt.float32, kind="ExternalInput")
    w = nc.dram_tensor("w", [128, H], mybir.dt.float32, kind="ExternalInput")
    bt_d = nc.dram_tensor("bt", [H, 1], mybir.dt.float32, kind="ExternalInput")
    sc_d = nc.dram_tensor("sc", [H, 1], mybir.dt.float32, kind="ExternalInput")
    sh_d = nc.dram_tensor("sh", [H, 1], mybir.dt.float32, kind="ExternalInput")
    out = nc.dram_tensor("out", [H, EPC], mybir.dt.float32, kind="ExternalOutput")

    f32 = mybir.dt.float32
    AF = mybir.ActivationFunctionType
    AO = mybir.AluOpType
    with TileContext(nc) as tc:
        with tc.tile_pool(name="io", bufs=3) as io, \
             tc.tile_pool(name="wp", bufs=1) as wp, \
             tc.tile_pool(name="ps", bufs=8, space="PSUM") as ps, \
             tc.tile_pool(name="zb", bufs=2) as zbp, \
             tc.tile_pool(name="ez", bufs=2) as ezp, \
             tc.tile_pool(name="op", bufs=3) as op:
            wt = wp.tile([128, H], f32)
            nc.sync.dma_start(out=wt[:], in_=w[:, :])
            btt = wp.tile([H, 1], f32)
            nc.sync.dma_start(out=btt[:], in_=bt_d[:, :])
            sct = wp.tile([H, 1], f32)
            nc.sync.dma_start(out=sct[:], in_=sc_d[:, :])
            sht = wp.tile([H, 1], f32)
            nc.sync.dma_start(out=sht[:], in_=sh_d[:, :])

            for blk in range(EPC // COLS):      # 30 blocks
                it = io.tile([128, COLS], f32)
                nc.sync.dma_start(out=it[:], in_=msg[:, blk*COLS:(blk+1)*COLS])
                zt = zbp.tile([H, COLS], f32)
                et = ezp.tile([H, COLS], f32)
                for m in range(COLS // TILE):   # 10 matmuls
                    pt = ps.tile([H, TILE], f32)
                    nc.tensor.matmul(
                        out=pt[:], lhsT=wt[:],
                        rhs=it[:, m*TILE:(m+1)*TILE],
                        start=True, stop=True)
                    sl = slice(m*TILE, (m+1)*TILE)
                    # relu(z+b) and exp(z+b) from PSUM
                    nc.scalar.activation(out=zt[:, sl], in_=pt[:], func=AF.Relu,
                                         bias=btt[:], scale=1.0)
                    nc.scalar.activation(out=et[:, sl], in_=pt[:], func=AF.Exp,
                                         bias=btt[:], scale=1.0)
                # elu(z) = relu(z) + min(exp(z)-1, 0); then *sc + sh
                ot = op.tile([H, COLS], f32)
                nc.vector.tensor_scalar(out=et[:], in0=et[:],
                                        scalar1=-1.0, scalar2=0.0,
                                        op0=AO.add, op1=AO.min)
                nc.vector.tensor_tensor(out=ot[:], in0=zt[:], in1=et[:],
                                        op=AO.add)
                nc.vector.tensor_scalar(out=ot[:], in0=ot[:],
                                        scalar1=sct[:], scalar2=sht[:],
                                        op0=AO.mult, op1=AO.add)
                nc.sync.dma_start(out=out[:, blk*COLS:(blk+1)*COLS], in_=ot[:])
    nc.compile()
    return nc


def _edge_layer_device(msgT, w_np, b_np, sc_np, sh_np):
    """msgT: [128, E] f32 feature-major messages. Returns [E, 64] f32."""
    if "nc" not in _cache:
        _cache["nc"] = _build_nc()
    nc = _cache["nc"]
    in_maps = [{
        "msg": np.ascontiguousarray(msgT[:, c*EPC:(c+1)*EPC]),
        "w": np.ascontiguousarray(w_np, dtype=np.float32),
        "bt": b_np.reshape(H, 1).astype(np.float32),
        "sc": sc_np.reshape(H, 1).astype(np.float32),
        "sh": sh_np.reshape(H, 1).astype(np.float32),
    } for c in range(NCORES)]
    res = run_bass_kernel_spmd(nc, in_maps, core_ids=list(range(NCORES)))
    return np.concatenate(
        [res.results[c]["out"].T for c in range(NCORES)], axis=0)  # [E, 64]


def _elu(x):
    return np.where(x > 0, x, np.expm1(np.minimum(x, 0.0)))


def kernel(x, edge_index, batch,
           enc_w1, enc_b1, enc_w2, enc_b2,
           c1_w, c1_b, c1_g, c1_be, c1_m, c1_v,
           c2_w, c2_b, c2_g, c2_be, c2_m, c2_v,
           c3_w, c3_b, c3_g, c3_be, c3_m, c3_v,
           out_w1, out_b1, out_w2, out_b2, out_w3, out_b3):
    x = np.asarray(x, dtype=np.float32)
    edge_index = np.asarray(edge_index)
    batch = np.asarray(batch)
    src = edge_index[0].astype(np.int64)
    dst = edge_index[1].astype(np.int64)

    to32 = lambda a: np.asarray(a, dtype=np.float32)

    # encoder (host; ~0.1% of total flops)
    h = _elu(_elu(x @ to32(enc_w1) + to32(enc_b1)) @ to32(enc_w2) + to32(enc_b2))

    cnt = np.bincount(dst, minlength=N).astype(np.float32)
    cnt = np.maximum(cnt, 1.0)[:, None]

    def edge_conv(feat, w, b, g, be, m, v):
        w = to32(w); b = to32(b); g = to32(g)
        be = to32(be); m = to32(m); v = to32(v)
        scale = g / np.sqrt(v + BN_EPS)
        shift = be - m * scale
        xi = feat[dst]
        msgT = np.empty((128, E), dtype=np.float32)
        msgT[:H] = xi.T
        msgT[H:] = (feat[src] - xi).T
        hbn = _edge_layer_device(msgT, w, b, scale, shift)  # [E, 64]
        s = np.zeros((N, H), dtype=np.float32)
        np.add.at(s, dst, hbn)
        return s / cnt

    f1 = edge_conv(h, c1_w, c1_b, c1_g, c1_be, c1_m, c1_v)
    f2 = edge_conv(f1 + h, c2_w, c2_b, c2_g, c2_be, c2_m, c2_v)
    f3 = edge_conv(f2 + f1, c3_w, c3_b, c3_g, c3_be, c3_m, c3_v)

    o = _elu(f3 @ to32(out_w1) + to32(out_b1))
    o = _elu(o @ to32(out_w2) + to32(out_b2))
    o = o @ to32(out_w3) + to32(out_b3)
    return (o, batch)
